# revision 17
# baseline (speedup 1.0000x reference)
"""Trainium2 Bass kernel for nn_MultiHeadAttention (B=2, S=2048, D=1024, H=16).

Sharding (8 cores): data-parallel over batch (2) x tensor-parallel over
head groups (4 groups of 4 heads). Core c handles batch c//4, heads
4*(c%4) .. 4*(c%4)+3. Each core computes full attention for its heads
plus its slice of the output projection; the host sums the 4 partial
output projections per batch and adds bo.

Host-side input layout: each of Q/K/V is sent as a [128, 16384] bf16
"quarter" layout (quarter q holds seq cols q*512..+512 for all 8
contraction tiles side by side), so one dma_start loads 1 MB with 8 KB
descriptors. Weights are tiled the same way. This keeps DMA issue
count tiny (the DGE setup of ~1.2us/start was the old startup
bottleneck) and lets the first projection matmuls start ~6us in.

On-chip: qT/kT [feat, seq] bf16, v [keys, 4*(64+1)] with a ones column
(row 64 of the ctx accumulation = sum of exp per query), scores ->
exp(scale fused) on the ACT engine, ctx accumulated over key tiles in
PSUM. Normalization: reciprocal of the sum row (DVE) -> broadcast to
64 partitions via a 1-contraction PE matmul -> multiply (DVE). All
matmuls bf16 with fp32 PSUM.

Schedule: projections not needed up front are self-contained filler
units popped one per exp-chunk so the tensor engine never idles (PE
idle gaps also drop its DVFS clock from 2.4 to ~1.2 GHz, which was
the main cost of the previous version). PSUM: 2x[128,1024] score
bufs, 2x[65,512] ctx accumulators, 2x[128,512] general bufs.
"""

import sys

for _p in ("/opt/trn_rl_repo",):
    if _p not in sys.path:
        sys.path.insert(0, _p)

from contextlib import ExitStack

import ml_dtypes
import numpy as np

import concourse.bass as bass
import concourse.tile as tile
from concourse import bacc, mybir
from concourse.bass_utils import run_bass_kernel_spmd

B, S, D, H = 2, 2048, 1024, 16
HD = D // H            # 64 head dim
NG = 4                 # head groups (cores per batch)
NHC = H // NG          # 4 heads per core
FS = NHC * HD          # 256 features per core
P = 128
DK = D // P            # 8 contraction tiles for projections
SK = S // P            # 16 key tiles
NQ = S // 512          # 4 query chunks
FK = FS // P           # 2 feature tiles for qT/kT/ctx
VW = HD + 1            # v feats + ones column
QW = DK * 512          # 4096: input quarter width

f32 = mybir.dt.float32
bf16 = mybir.dt.bfloat16
EXP = mybir.ActivationFunctionType.Exp
NCH = 8                # exp chunks (2 key tiles each) per head pass
N_WARM = 40            # PE pre-warm matmuls


def _emit(ctx: ExitStack, tc, nc, io):
    KQ_d, QQ_d, VQ_d, WqT, WkT, WvT, WoT, bq, bk, bv, OUTP, SCR = io

    wp = ctx.enter_context(tc.tile_pool(name="wp", bufs=1))
    xin = ctx.enter_context(tc.tile_pool(name="xin", bufs=1))
    per = ctx.enter_context(tc.tile_pool(name="per", bufs=1))
    exp = ctx.enter_context(tc.tile_pool(name="exp", bufs=12))
    nrm = ctx.enter_context(tc.tile_pool(name="nrm", bufs=2))
    ctxp = ctx.enter_context(tc.tile_pool(name="ctxp", bufs=2))
    outp = ctx.enter_context(tc.tile_pool(name="outp", bufs=4))
    sc_ps = ctx.enter_context(tc.tile_pool(name="sc_ps", bufs=2, space="PSUM"))
    cx_ps = ctx.enter_context(tc.tile_pool(name="cx_ps", bufs=2, space="PSUM"))
    gn_ps = ctx.enter_context(tc.tile_pool(name="gn_ps", bufs=2, space="PSUM"))

    # ---- DMA: few big transfers, critical path (K, Wk, Q, Wq) first ----
    QQ = [xin.tile([P, QW], bf16, tag=f"qq{q}", name=f"qqt{q}") for q in range(4)]
    wk = wp.tile([P, DK * FS], bf16, tag="wk")
    nc.sync.dma_start(wk[:], WkT[:, :])
    KQ = [xin.tile([P, QW], bf16, tag=f"kq{q}", name=f"kqt{q}") for q in range(4)]
    for s in range(4):
        nc.sync.dma_start(KQ[0][:, s * 1024:(s + 1) * 1024],
                          KQ_d[:, s * 1024:(s + 1) * 1024])
    for q in range(1, 4):
        nc.sync.dma_start(KQ[q][:], KQ_d[:, q * QW:(q + 1) * QW])
    for q in range(1, 4):
        nc.sync.dma_start(QQ[q][:], QQ_d[:, q * QW:(q + 1) * QW])
    wo = wp.tile([P, FK * D], bf16, tag="wo")
    nc.sync.dma_start(wo[:], WoT[:, :])

    wq = wp.tile([P, DK * FS], bf16, tag="wq")
    nc.scalar.dma_start(wq[:], WqT[:, :])
    for s in range(4):
        nc.scalar.dma_start(QQ[0][:, s * 1024:(s + 1) * 1024],
                            QQ_d[:, s * 1024:(s + 1) * 1024])
    bq_t = wp.tile([P, FK], f32, tag="bq")
    nc.gpsimd.dma_start(bq_t[:], bq[:, :])
    bk_t = wp.tile([P, FK], f32, tag="bk")
    nc.gpsimd.dma_start(bk_t[:], bk[:, :])

    wv = wp.tile([P, DK * FS], bf16, tag="wv")
    VQ = [xin.tile([P, QW], bf16, tag=f"vq{q}", name=f"vqt{q}") for q in range(4)]
    bv_t = wp.tile([P, FS], f32, tag="bv")

    ones_t = wp.tile([P, NHC], f32, tag="ones")
    nc.vector.memset(ones_t[:], 1.0)

    # ---- persistent activations ----
    kT = [per.tile([P, S], bf16, tag=f"kT{f}", name=f"kTs{f}") for f in range(FK)]
    qT = [per.tile([P, S], bf16, tag=f"qT{f}", name=f"qTs{f}") for f in range(FK)]
    vsb = [per.tile([P, NHC * VW], bf16, tag=f"v{t}", name=f"vs{t}") for t in range(SK)]

    # PE pre-warm (ramps DVFS) + Exp table preload while first DMAs land.
    warm_sb = wp.tile([P, 16], f32, tag="warm")
    nc.vector.memset(warm_sb[:], 0.0)
    warm_ps = gn_ps.tile([16, 16], f32, tag="gen", name="warmps")
    for _ in range(N_WARM):
        nc.tensor.matmul(warm_ps[:], warm_sb[:, 0:16], warm_sb[:],
                         start=True, stop=True)
    warm_ex = wp.tile([P, 16], bf16, tag="warmex")
    nc.scalar.activation(warm_ex[:], warm_sb[:], EXP, scale=0.125)

    # ---- projection emitters (self-contained units) ----
    def kq_proj(XQ, w, b_t, dst, ncol, f):
        ps = gn_ps.tile([P, 512], f32, tag="gen", name="kqps")
        for k in range(DK):
            nc.tensor.matmul(
                ps[:], w[:, k * FS + f * P:k * FS + (f + 1) * P],
                XQ[ncol][:, k * 512:(k + 1) * 512],
                start=(k == 0), stop=(k == DK - 1))
        nc.vector.tensor_scalar_add(
            dst[f][:, ncol * 512:(ncol + 1) * 512], ps[:], b_t[:, f:f + 1])

    def kq_unit(XQ, w, b_t, dst, ncol, f):
        return lambda: kq_proj(XQ, w, b_t, dst, ncol, f)

    def vproj_unit(t):
        def emit():
            ps = gn_ps.tile([P, FS], f32, tag="gen", name="vps")
            qt, co = divmod(t, 4)
            for k in range(DK):
                nc.tensor.matmul(
                    ps[:], VQ[qt][:, k * 512 + co * P:k * 512 + co * P + P],
                    wv[:, k * FS:(k + 1) * FS],
                    start=(k == 0), stop=(k == DK - 1))
            for h in range(NHC):
                nc.vector.tensor_add(
                    vsb[t][:, h * VW:h * VW + HD],
                    ps[:, h * HD:(h + 1) * HD],
                    bv_t[:, h * HD:(h + 1) * HD])
            nc.vector.tensor_copy(vsb[t][:, HD:NHC * VW:VW], ones_t[:])
        return emit

    def outproj_unit(jj, ctxn, mt):
        def emit():
            ob = outp.tile([P, D], bf16, tag="ob", name="ob")
            for oc in range(2):
                ps = gn_ps.tile([P, 512], f32, tag="gen", name="ops")
                for f in range(FK):
                    nc.tensor.matmul(
                        ps[:], ctxn[f][:, mt * P:(mt + 1) * P],
                        wo[:, f * D + oc * 512:f * D + (oc + 1) * 512],
                        start=(f == 0), stop=(f == FK - 1))
                nc.vector.tensor_copy(ob[:, oc * 512:(oc + 1) * 512], ps[:])
            nc.sync.dma_start(
                OUTP[jj * 512 + mt * P:jj * 512 + (mt + 1) * P, :], ob[:])
        return emit

    # ---- startup projections (critical path for chunk 0..3, heads 0/1) ----
    kq_proj(KQ, wk, bk_t, kT, 0, 0)
    kq_proj(QQ, wq, bq_t, qT, 0, 0)
    kq_proj(KQ, wk, bk_t, kT, 1, 0)

    # Bulk loads gated behind the critical path: each queue first stores one
    # element of a freshly projected tile, so its DGE waits until the
    # startup projections' inputs have landed before eating HBM bandwidth.
    nc.scalar.dma_start(SCR[0:1, 0:1], kT[0][0:1, 0:1])
    nc.scalar.dma_start(wv[:], WvT[:, :])
    nc.scalar.dma_start(VQ[0][:], VQ_d[:, 0:QW])
    nc.scalar.dma_start(bv_t[:], bv[:, :])
    for q in range(1, 4):
        nc.scalar.dma_start(VQ[q][:], VQ_d[:, q * QW:(q + 1) * QW])


    def norm_head(h, cp, ctxn):
        fq, rq = divmod(h * HD, P)
        sm = nrm.tile([1, 512], f32, tag="sm", name="sm")
        nc.vector.tensor_copy(sm[:], cp[HD:HD + 1, :])
        rb = nrm.tile([1, 512], f32, tag="rb", name="rb")
        nc.vector.reciprocal_approx_fast(rb[:], sm[:])
        rbb = nrm.tile([HD, 512], f32, tag="rbb", name="rbb")
        nc.gpsimd.partition_broadcast(rbb[:], rb[:])
        nc.vector.tensor_mul(ctxn[fq][rq:rq + HD, :], cp[0:HD, :], rbb[:])

    prev = None
    for j in range(NQ):
        units = []
        if j == 0:
            kA = [kq_unit(KQ, wk, bk_t, kT, 2, 0),   # needed chunk 4
                  kq_unit(KQ, wk, bk_t, kT, 3, 0),   # chunk 6
                  kq_unit(KQ, wk, bk_t, kT, 0, 1),   # chunk 16 (heads 2/3)
                  kq_unit(KQ, wk, bk_t, kT, 1, 1),   # chunk 18
                  kq_unit(KQ, wk, bk_t, kT, 2, 1),   # chunk 20
                  kq_unit(KQ, wk, bk_t, kT, 3, 1),   # chunk 22
                  kq_unit(QQ, wq, bq_t, qT, 0, 1)]   # chunk 16
            q1 = [kq_unit(QQ, wq, bq_t, qT, 1, 0),   # j1 (Qq1 lands ~33us)
                  kq_unit(QQ, wq, bq_t, qT, 1, 1)]
            vp = [vproj_unit(t) for t in range(SK)]
            units = (kA + vp[0:6] + q1[0:1] + vp[6:8] + q1[1:2] + vp[8:16])
            is_v = ([False] * 7 + [True] * 6 + [False] + [True] * 2 +
                    [False] + [True] * 8)
            # vp[8:16] pop 2/chunk so head 0's last ctx (and the norm
            # that opens the cx gate) lands by ~chunk 21, not 24
            targets = (list(range(17)) +
                       [17, 17, 18, 18, 19, 19, 20, 20])
        else:
            is_v = None
            if j <= 2:
                for f in range(FK):
                    units.append(kq_unit(QQ, wq, bq_t, qT, j + 1, f))
            for mt in range(4):
                units.append(outproj_unit(prev[0], prev[1], mt))
            targets = [6 + 3 * i for i in range(len(units))]

        qi = 0
        n_v = 0
        ctxn = [ctxp.tile([P, 512], bf16, tag=f"ctxn{f}", name=f"cn{f}")
                for f in range(FK)]
        ctx_ps_h = {}
        exb = {}                      # (h, c) -> ex tile
        next_c = [0] * NHC            # per-head next ctx chunk to emit
        emitted = [0] * NHC           # per-head #score-chunks emitted
        normed = [0]                  # heads fully normalized this pass

        def emit_ctx(h):
            c = next_c[h]
            if h not in ctx_ps_h:
                ctx_ps_h[h] = cx_ps.tile([VW, 512], f32, tag="cx",
                                         name=f"cps{h}")
            cp = ctx_ps_h[h]
            ex = exb.pop((h, c))
            for t in range(2):
                kt2 = 2 * c + t
                nc.tensor.matmul(
                    cp[:], vsb[kt2][:, h * VW:(h + 1) * VW],
                    ex[:, t * 512:(t + 1) * 512],
                    start=(kt2 == 0), stop=(kt2 == SK - 1))
            next_c[h] += 1
            if next_c[h] == NCH:
                norm_head(h, cp, ctxn)
                normed[0] += 1

        def drain():
            # emit any ctx whose v tiles are ready (j>0: all ready); a head
            # may only have an open accumulator if the head two below is
            # fully normalized (cx_ps ring has 2 banks)
            tmax = n_v - 1 if j == 0 else 10 ** 9
            for h in range(NHC):
                if h >= normed[0] + 2:
                    break
                while next_c[h] < emitted[h] and 2 * next_c[h] + 1 <= tmax:
                    emit_ctx(h)

        for h in range(NHC):
            fq, rq = divmod(h * HD, P)
            qv = qT[fq][rq:rq + HD, j * 512:(j + 1) * 512]
            for c in range(NCH):
                sc = sc_ps.tile([P, 2 * 512], f32, tag="sc", name="sc")
                for t in range(2):
                    kt2 = 2 * c + t
                    nc.tensor.matmul(
                        sc[:, t * 512:(t + 1) * 512],
                        kT[fq][rq:rq + HD, kt2 * P:(kt2 + 1) * P],
                        qv, start=True, stop=True)
                ex = exp.tile([P, 2 * 512], bf16, tag="ex", name="ex")
                nc.scalar.activation(ex[:], sc[:], EXP,
                                     scale=1.0 / (HD ** 0.5))
                exb[(h, c)] = ex
                emitted[h] += 1
                g = h * NCH + c
                while qi < len(units) and targets[qi] <= g:
                    units[qi]()
                    if is_v is not None and is_v[qi]:
                        n_v += 1
                    qi += 1
                drain()
        while qi < len(units):
            units[qi]()
            if is_v is not None and is_v[qi]:
                n_v += 1
            qi += 1
        drain()
        prev = (j, ctxn)
    for mt in range(4):
        outproj_unit(prev[0], prev[1], mt)()


_CACHE = {}


def _build():
    if "nc" in _CACHE:
        return _CACHE["nc"]
    nc = bacc.Bacc("TRN2", target_bir_lowering=False, debug=False)
    KQ_d = nc.dram_tensor("KQ", [P, 4 * QW], bf16, kind="ExternalInput").ap()
    QQ_d = nc.dram_tensor("QQ", [P, 4 * QW], bf16, kind="ExternalInput").ap()
    VQ_d = nc.dram_tensor("VQ", [P, 4 * QW], bf16, kind="ExternalInput").ap()
    WqT = nc.dram_tensor("WqT", [P, DK * FS], bf16, kind="ExternalInput").ap()
    WkT = nc.dram_tensor("WkT", [P, DK * FS], bf16, kind="ExternalInput").ap()
    WvT = nc.dram_tensor("WvT", [P, DK * FS], bf16, kind="ExternalInput").ap()
    WoT = nc.dram_tensor("WoT", [P, FK * D], bf16, kind="ExternalInput").ap()
    bq = nc.dram_tensor("bq", [P, FK], f32, kind="ExternalInput").ap()
    bk = nc.dram_tensor("bk", [P, FK], f32, kind="ExternalInput").ap()
    bv = nc.dram_tensor("bv", [P, FS], f32, kind="ExternalInput").ap()
    OUTP = nc.dram_tensor("OUTP", [S, D], bf16, kind="ExternalOutput").ap()
    SCR = nc.dram_tensor("SCR", [1, 8], bf16, kind="ExternalOutput").ap()
    with tile.TileContext(nc) as tc, ExitStack() as ctx:
        _emit(ctx, tc, nc, (KQ_d, QQ_d, VQ_d, WqT, WkT, WvT, WoT,
                            bq, bk, bv, OUTP, SCR))
    nc.compile()
    _CACHE["nc"] = nc
    return nc


def _quarters(x):
    # x: [S, D] -> [128, 16384] bf16; [p, q*4096+k*512+c] = x.T[k*128+p, q*512+c]
    a = np.asarray(x, np.float32).T
    r = a.reshape(DK, P, 4, 512).transpose(1, 2, 0, 3).reshape(P, 4 * QW)
    return np.ascontiguousarray(r).astype(ml_dtypes.bfloat16)


def _wlay(w):
    # w: [FS, D] (torch Linear rows) -> [128, DK*FS]; block k = W.T rows k*128..
    a = np.asarray(w, np.float32).T
    r = a.reshape(DK, P, FS).transpose(1, 0, 2).reshape(P, DK * FS)
    return np.ascontiguousarray(r).astype(ml_dtypes.bfloat16)


def _wolay(w):
    # w: Wo[:, sl] [D, FS] -> WoT [FS, D] -> [128, FK*D]
    a = np.asarray(w, np.float32).T
    r = a.reshape(FK, P, D).transpose(1, 0, 2).reshape(P, FK * D)
    return np.ascontiguousarray(r).astype(ml_dtypes.bfloat16)


def _in_maps(Q, K, V, Wq, bq, Wk, bk, Wv, bv, Wo, bo):
    QQb = [_quarters(Q[b]) for b in range(B)]
    KQb = [_quarters(K[b]) for b in range(B)]
    VQb = [_quarters(V[b]) for b in range(B)]
    maps = []
    for core in range(8):
        b, g = divmod(core, NG)
        sl = slice(g * FS, (g + 1) * FS)
        maps.append({
            "QQ": QQb[b], "KQ": KQb[b], "VQ": VQb[b],
            "WqT": _wlay(Wq[sl, :]), "WkT": _wlay(Wk[sl, :]),
            "WvT": _wlay(Wv[sl, :]), "WoT": _wolay(Wo[:, sl]),
            "bq": np.ascontiguousarray(
                np.asarray(bq, np.float32)[sl].reshape(FK, P).T),
            "bk": np.ascontiguousarray(
                np.asarray(bk, np.float32)[sl].reshape(FK, P).T),
            "bv": np.ascontiguousarray(np.broadcast_to(
                np.asarray(bv, np.float32)[sl], (P, FS))),
        })
    return maps


def kernel(Q, K, V, Wq, bq, Wk, bk, Wv, bv, Wo, bo):
    nc = _build()
    maps = _in_maps(Q, K, V, Wq, bq, Wk, bk, Wv, bv, Wo, bo)
    res = run_bass_kernel_spmd(nc, maps, core_ids=list(range(8)))
    out = np.empty((B, S, D), np.float32)
    for b in range(B):
        acc = res.results[b * NG]["OUTP"].astype(np.float32)
        for g in range(1, NG):
            acc = acc + res.results[b * NG + g]["OUTP"].astype(np.float32)
        out[b] = acc + np.asarray(bo, np.float32)[None, :]
    return out
